# Initial kernel scaffold
#
"""Causal self-attention (GQA + RoPE) Trainium2 kernel over 8 NeuronCores.

Sharding: 8 cores = batch(2) x kv_head(4). Each core computes its batch's
4 q-heads / 1 kv-head attention plus the partial output projection; host
sums the 4 partial projections per batch element.

Device algorithm (fully transposed "k-major" attention, zero P-transposes):
  stage A:  QKV^T = [wq|wk|wv]^T @ x^T on PE (bf16, fp32 accum); the Q1
            (heads 2,3) part is interleaved into attention pass 0 as PE
            filler so the tensor engine never idles (keeps HAM at 2.4GHz)
  RoPE on DVE via interleaved-pair layout (host permutes wq/wk columns so
    rotation partners are adjacent partitions -> stream_shuffle swap)
  V^T -> V via PE transposes; ones-column appended to V so the PV matmul
    emits both Y^T and the softmax sums in one accumulation
  attention in two head-pair passes (frees PSUM banks); S^T = K^T.T @ Q^T,
    one exp ACTIVATE per two S tiles (exp is elementwise; per-ik partition
    semantics only matter to the PV consumer), causal staircase trimming,
    single tri-mask multiply per diagonal block
  normalize: fast reciprocal + gpsimd partition_broadcast + DVE mul, off
    the critical path via an eager PSUM->SBUF copy
  projection: wproj rows for this core's heads (bf16), interleaved into
    pass 1; partial f32 out to HBM
"""

import sys

sys.path.insert(0, "/opt/trn_rl_repo")

import numpy as np
import ml_dtypes

import concourse.bacc as bacc
import concourse.tile as tile
from concourse import mybir
from concourse.bass_utils import run_bass_kernel_spmd

F32 = mybir.dt.float32
BF16 = mybir.dt.bfloat16
AF = mybir.ActivationFunctionType

T, C, D, H, HKV = 2048, 1024, 64, 16, 4
G = H // HKV  # q heads per kv head
NCC = C // 128  # 8 contraction chunks
NJQ = 4  # tq chunks of 512
TQC = 512
NTK = T // 128  # 16 tk tiles
SCALE = 1.0 / 8.0  # 1/sqrt(D)

_PROG = {}


def _build_program():
    nc = bacc.Bacc()
    xT_d = nc.dram_tensor("xT", [C, T], BF16, kind="ExternalInput")
    w_d = nc.dram_tensor("w_all", [C, 384], BF16, kind="ExternalInput")
    wp_d = nc.dram_tensor("wp", [256, C], BF16, kind="ExternalInput")
    cq_d = nc.dram_tensor("cos_q", [128, T], F32, kind="ExternalInput")
    sq_d = nc.dram_tensor("sin_q", [128, T], F32, kind="ExternalInput")
    mk_d = nc.dram_tensor("masks", [128, 128], BF16, kind="ExternalInput")
    id_d = nc.dram_tensor("identb", [128, 128], F32, kind="ExternalInput")
    out_d = nc.dram_tensor("out_p", [T, C], F32, kind="ExternalOutput")

    swap_mask = [i ^ 1 for i in range(32)]

    with tile.TileContext(nc) as tc:
        with (
            tc.tile_pool(name="const", bufs=1) as const,
            tc.tile_pool(name="big", bufs=1) as big,
            tc.tile_pool(name="xp", bufs=1) as xp,
            tc.tile_pool(name="rope", bufs=1) as ropep,
            tc.tile_pool(name="ptiles", bufs=8) as ppool,
            tc.tile_pool(name="small", bufs=3) as small,
            tc.tile_pool(name="outp", bufs=3) as outp,
        ):
            W_sb = const.tile([128, NCC, 384], BF16, tag="W", name="W_sb")
            cq_sb = const.tile([128, T], F32, tag="cq", name="cq_sb")
            sq_sb = const.tile([128, T], F32, tag="sq", name="sq_sb")
            mk_sb = const.tile([128, 128], BF16, tag="mk", name="mk_sb")
            id_sb = const.tile([128, 128], F32, tag="idb", name="id_sb")
            wp_sb = const.tile([128, 2, C], BF16, tag="wp", name="wp_sb")

            qkv_sb = [big.tile([128, T], F32, tag=f"qkv{m}", name=f"qkv{m}") for m in range(3)]
            qrope = [big.tile([128, T], BF16, tag=f"qr{m}", name=f"qr{m}") for m in range(2)]
            k2 = big.tile([128, T], BF16, tag="k2", name="k2")
            vhat = big.tile([128, NTK, 65], BF16, tag="vhat", name="vhat")
            yn = [big.tile([128, T], BF16, tag=f"yn{m}", name=f"yn{m}") for m in range(2)]

            xts = []
            for cc in range(NCC):
                xt = xp.tile([128, T], BF16, tag=f"x{cc}", name=f"x{cc}")
                nc.sync.dma_start(out=xt[:], in_=xT_d[cc * 128 : (cc + 1) * 128, :])
                nc.sync.dma_start(out=W_sb[:, cc, :], in_=w_d[cc * 128 : (cc + 1) * 128, :])
                xts.append(xt)
            nc.sync.dma_start(out=cq_sb[:], in_=cq_d[:])
            nc.sync.dma_start(out=sq_sb[:], in_=sq_d[:])
            nc.sync.dma_start(out=mk_sb[:], in_=mk_d[:])
            nc.sync.dma_start(out=id_sb[:], in_=id_d[:])
            nc.sync.dma_start(out=wp_sb[:], in_=wp_d[:].rearrange("(n p) m -> p n m", p=128))

            def emit_rope(pt, jqs=None):
                rows = 128 if pt < 2 else 64
                dst = qrope[pt] if pt < 2 else k2
                src = qkv_sb[pt]
                for j in jqs if jqs is not None else range(NJQ):
                    cs = slice(j * TQC, (j + 1) * TQC)
                    shuf = ropep.tile([128, TQC], F32, tag="shuf", name="shuf")
                    prod = ropep.tile([128, TQC], F32, tag="prod", name="prod")
                    nc.vector.stream_shuffle(shuf[:rows, :], src[:rows, cs], mask=swap_mask)
                    nc.vector.tensor_mul(out=shuf[:rows, :], in0=shuf[:rows, :], in1=sq_sb[:rows, cs])
                    nc.vector.tensor_mul(out=prod[:rows, :], in0=src[:rows, cs], in1=cq_sb[:rows, cs])
                    nc.vector.tensor_add(out=dst[:rows, cs], in0=prod[:rows, :], in1=shuf[:rows, :])

            # ---- stage A part 1: KV + Q0 projections, rope, Vhat ----
            with (
                tc.tile_pool(name="psA", bufs=1, space="PSUM") as psA,
                tc.tile_pool(name="psT", bufs=2, space="PSUM") as psT,
            ):
                # warmup: garbage matmuls with no input deps keep the PE busy
                # through the initial DMA window so HAM reaches 2.4GHz before
                # stage A starts (values never read; psum overwritten later)
                wu = ropep.tile([128, 512], BF16, tag="wu", name="wu")
                nc.vector.memset(wu[:], 0.5)
                puw = psT.tile([128, 512], F32, tag="ptr", name="puw")
                for _ in range(24):
                    nc.tensor.matmul(puw[:], lhsT=wu[:, 0:128], rhs=wu[:], start=True, stop=True)
                for mt in (2, 0):
                    pas = [psA.tile([128, TQC], F32, tag=f"pa{j}", name=f"pa{j}") for j in range(NJQ)]
                    for cc in range(NCC):
                        for jq in range(NJQ):
                            nc.tensor.matmul(
                                pas[jq][:],
                                lhsT=W_sb[:, cc, mt * 128 : (mt + 1) * 128],
                                rhs=xts[cc][:, jq * TQC : (jq + 1) * TQC],
                                start=(cc == 0),
                                stop=(cc == NCC - 1),
                            )
                    for jq in range(NJQ):
                        nc.scalar.copy(
                            out=qkv_sb[mt][:, jq * TQC : (jq + 1) * TQC], in_=pas[jq][:]
                        )

                emit_rope(2)  # K first: attention depends on it
                # duplicate K^T into partitions 64:128 (head-pair row groups)
                nc.sync.dma_start(out=k2[64:128, :], in_=k2[0:64, :])
                emit_rope(0, jqs=(3, 2, 1, 0))

                # Vhat: V (t-major) + ones column for softmax sums
                nc.vector.memset(vhat[:, :, 64:65], 1.0)
                for tt in range(NTK):
                    pt_ = psT.tile([128, 64], F32, tag="ptr", name="ptr")
                    nc.tensor.transpose(
                        pt_[:],
                        qkv_sb[2][64:128, tt * 128 : (tt + 1) * 128],
                        id_sb[64:128, 0:64],
                    )
                    nc.vector.tensor_copy(out=vhat[:, tt, 0:64], in_=pt_[:])

            def attention_pass(hp, psS, psY, filler, boundary, jq_order=tuple(range(NJQ))):
                """One head-pair pass. filler() emits one unit of extra PE work
                (stage-A Q1 / projection) per group to keep the PE dense;
                boundary(jq) runs after each chunk before the normalize."""
                for jq in jq_order:
                    nik = 4 * jq + 4
                    pys = [psY.tile([65, TQC], F32, tag="py", name="py") for _ in range(2)]
                    for ika in range(0, nik, 2):
                        iks = (ika, ika + 1)
                        filler()
                        qt = qrope[hp]
                        los = []
                        for gi, ik in enumerate(iks):
                            s = ik - 4 * jq
                            los.append(max(s, 0) * 128)
                        ps_gs = [
                            psS.tile([128, 2, TQC], F32, tag=f"ps_g{hh}", name=f"ps_g{hh}")
                            for hh in range(2)
                        ]
                        # alternate row groups (hh base 0 / 64) so consecutive
                        # half-array S matmuls overlap in the PE array
                        for gi, ik in enumerate(iks):
                            lo = los[gi]
                            for hh in range(2):
                                base = hh * 64
                                nc.tensor.matmul(
                                    ps_gs[hh][:, gi, lo:TQC],
                                    lhsT=k2[base : base + 64, ik * 128 : (ik + 1) * 128],
                                    rhs=qt[base : base + 64, jq * TQC + lo : (jq + 1) * TQC],
                                    start=True,
                                    stop=True,
                                )
                        mlo = min(los)
                        ptiles = []
                        for hh in range(2):
                            ptile = ppool.tile([128, 2, TQC], BF16, tag="pt", name="ptile")
                            nc.scalar.activation(
                                out=ptile[:, :, mlo:TQC],
                                in_=ps_gs[hh][:, :, mlo:TQC],
                                func=AF.Exp,
                                scale=SCALE,
                            )
                            ptiles.append(ptile)
                        for hh in range(2):
                            for gi, ik in enumerate(iks):
                                if ik - 4 * jq >= 0:
                                    lo = los[gi]
                                    nc.vector.tensor_mul(
                                        out=ptiles[hh][:, gi, lo : lo + 128],
                                        in0=ptiles[hh][:, gi, lo : lo + 128],
                                        in1=mk_sb[:, 0:128],
                                    )
                        for hh in range(2):
                            for gi, ik in enumerate(iks):
                                lo = los[gi]
                                nc.tensor.matmul(
                                    pys[hh][:, lo:TQC],
                                    lhsT=vhat[:, ik, :],
                                    rhs=ptiles[hh][:, gi, lo:TQC],
                                    start=(ik == 0),
                                    stop=(ik == nik - 1),
                                )
                    boundary(jq)
                    for hh in range(2):
                        # eager copy frees the PSUM accumulator; normalize runs
                        # off the critical path
                        ybuf = small.tile([65, TQC], F32, tag="ybuf", name="ybuf")
                        nc.vector.tensor_copy(out=ybuf[:], in_=pys[hh][:])
                        srow = small.tile([1, TQC], F32, tag="srow", name="srow")
                        nc.vector.tensor_copy(out=srow[:], in_=pys[hh][64:65, :])
                        rinv = small.tile([1, TQC], F32, tag="rinv", name="rinv")
                        nc.vector.reciprocal_approx_fast(out=rinv[:], in_=srow[:])
                        rb = small.tile([64, TQC], F32, tag="rb", name="rb")
                        nc.gpsimd.partition_broadcast(rb[:], rinv[:])
                        nc.vector.tensor_mul(
                            out=yn[hp][hh * 64 : hh * 64 + 64, jq * TQC : (jq + 1) * TQC],
                            in0=ybuf[0:64, :],
                            in1=rb[:],
                        )

            # ---- pass 0 (heads 0,1) with stage-A Q1 interleaved ----
            with (
                tc.tile_pool(name="psS0", bufs=1, space="PSUM") as psS0,
                tc.tile_pool(name="psY0", bufs=2, space="PSUM") as psY0,
                tc.tile_pool(name="psA2", bufs=2, space="PSUM") as psA2,
            ):

                def a2_gen():
                    for jq2 in range(NJQ):
                        pa = psA2.tile([128, TQC], F32, tag="pa2", name="pa2")
                        for cc in range(NCC):
                            nc.tensor.matmul(
                                pa[:],
                                lhsT=W_sb[:, cc, 128:256],
                                rhs=xts[cc][:, jq2 * TQC : (jq2 + 1) * TQC],
                                start=(cc == 0),
                                stop=(cc == NCC - 1),
                            )
                            yield
                        nc.vector.tensor_copy(
                            out=qkv_sb[1][:, jq2 * TQC : (jq2 + 1) * TQC], in_=pa[:]
                        )
                    emit_rope(1)
                    yield

                gen = a2_gen()

                def filler0():
                    next(gen, None)

                def filler0x2():
                    filler0()
                    filler0()

                # head start for the PE while rope-q0 finishes on DVE
                for _ in range(4):
                    filler0()
                attention_pass(
                    0, psS0, psY0, filler0x2, lambda jq: None, jq_order=(3, 2, 1, 0)
                )
                for _ in gen:
                    pass

            # ---- pass 1 (heads 2,3) with projection interleaved ----
            with (
                tc.tile_pool(name="psS1", bufs=1, space="PSUM") as psS1,
                tc.tile_pool(name="psY1", bufs=2, space="PSUM") as psY1,
                tc.tile_pool(name="psP", bufs=1, space="PSUM") as psP,
            ):

                def emit_proj(pjq):
                    for tt in range(4 * pjq, 4 * pjq + 4):
                        outsb = outp.tile([128, C], F32, tag="osb", name="osb")
                        pps = [psP.tile([128, 512], F32, tag=f"pp{n}", name=f"pp{n}") for n in range(2)]
                        for kk in range(2):
                            for ncol in range(2):
                                nc.tensor.matmul(
                                    pps[ncol][:],
                                    lhsT=yn[kk][:, tt * 128 : (tt + 1) * 128],
                                    rhs=wp_sb[:, kk, ncol * 512 : (ncol + 1) * 512],
                                    start=(kk == 0),
                                    stop=(kk == 1),
                                )
                        for ncol in range(2):
                            nc.vector.tensor_copy(
                                out=outsb[:, ncol * 512 : (ncol + 1) * 512], in_=pps[ncol][:]
                            )
                        nc.sync.dma_start(
                            out=out_d[tt * 128 : (tt + 1) * 128, :], in_=outsb[:]
                        )

                def boundary1(jq):
                    if jq > 0:
                        emit_proj(jq - 1)

                attention_pass(1, psS1, psY1, lambda: None, boundary1)
                emit_proj(NJQ - 1)

    nc.compile()
    return nc


def _host_tables():
    # RoPE tables in interleaved-pair device layout (row j'=2i <-> orig j=i,
    # j'=2i+1 <-> orig j=i+32); sign of the shuffled sin term folded in.
    inv = 1.0 / (10000.0 ** (np.arange(0, D, 2, dtype=np.float64) / D))  # (32,)
    t = np.arange(T, dtype=np.float64)
    fr = np.outer(t, inv)  # (T, 32)
    cos_h = np.cos(fr).T.astype(np.float32)  # (32, T)
    sin_h = np.sin(fr).T.astype(np.float32)
    cosI = np.empty((D, T), np.float32)
    sinI = np.empty((D, T), np.float32)
    cosI[0::2] = cos_h
    cosI[1::2] = cos_h
    sinI[0::2] = -sin_h
    sinI[1::2] = sin_h
    cos_q = np.tile(cosI, (2, 1))
    sin_q = np.tile(sinI, (2, 1))
    # tri mask for the diagonal 128-block: allowed iff tkl <= tql
    tkl = np.arange(128)[:, None]
    tql = np.arange(128)[None, :]
    mask = (tkl <= tql).astype(np.float32).astype(ml_dtypes.bfloat16)
    identb = np.tile(np.eye(64, dtype=np.float32), (2, 2))
    return cos_q, sin_q, mask, identb


def make_in_maps(x, wq, wk, wv, wproj):
    cos_q, sin_q, mask, identb = _host_tables()
    # interleave permutation within each head's 64 cols: perm[2i]=i, perm[2i+1]=i+32
    perm = np.empty(D, np.int64)
    perm[0::2] = np.arange(32)
    perm[1::2] = np.arange(32) + 32
    in_maps = []
    for c in range(8):
        b, h = c // 4, c % 4
        xT = np.ascontiguousarray(x[b].T).astype(ml_dtypes.bfloat16)  # (C, T)
        wq_h = wq[:, h * 256 : (h + 1) * 256].reshape(C, G, D)[:, :, perm].reshape(C, 256)
        wk_h = wk[:, h * 64 : (h + 1) * 64][:, perm]
        wv_h = wv[:, h * 64 : (h + 1) * 64]
        w_all = np.concatenate([wq_h, wk_h, wv_h], axis=1).astype(ml_dtypes.bfloat16)
        wp_h = wproj[h * 256 : (h + 1) * 256, :].astype(ml_dtypes.bfloat16)
        in_maps.append(
            {
                "xT": xT,
                "w_all": w_all,
                "wp": wp_h,
                "cos_q": cos_q,
                "sin_q": sin_q,
                "masks": mask,
                "identb": identb,
            }
        )
    return in_maps


def kernel(x, wq, wk, wv, wproj):
    x = np.asarray(x, dtype=np.float32)
    wq = np.asarray(wq, dtype=np.float32)
    wk = np.asarray(wk, dtype=np.float32)
    wv = np.asarray(wv, dtype=np.float32)
    wproj = np.asarray(wproj, dtype=np.float32)
    B = x.shape[0]

    if "nc" not in _PROG:
        _PROG["nc"] = _build_program()
    nc = _PROG["nc"]

    in_maps = make_in_maps(x, wq, wk, wv, wproj)

    res = run_bass_kernel_spmd(nc, in_maps, list(range(8)))
    out = np.zeros((B, T, C), np.float32)
    for c in range(8):
        out[c // 4] += res.results[c]["out_p"]
    return out



# revision 1
# speedup vs baseline: 1.0934x; 1.0934x over previous
"""Causal self-attention (GQA + RoPE) Trainium2 kernel over 8 NeuronCores.

Sharding: 8 cores = batch(2) x kv_head(4). Each core computes its batch's
4 q-heads / 1 kv-head attention plus the partial output projection; host
sums the 4 partial projections per batch element.

Device algorithm (fully transposed "k-major" attention, zero P-transposes):
  stage A:  QKV^T = [wq|wk|wv]^T @ x^T on PE (bf16, fp32 accum); the Q1
            (heads 2,3) part is interleaved into attention pass 0 as PE
            filler so the tensor engine never idles (keeps HAM at 2.4GHz)
  RoPE on DVE via interleaved-pair layout (host permutes wq/wk columns so
    rotation partners are adjacent partitions -> stream_shuffle swap)
  V^T -> V via PE transposes; ones-column appended to V so the PV matmul
    emits both Y^T and the softmax sums in one accumulation
  attention in two head-pair passes (frees PSUM banks); S^T = K^T.T @ Q^T,
    one exp ACTIVATE per two S tiles (exp is elementwise; per-ik partition
    semantics only matter to the PV consumer), causal staircase trimming,
    single tri-mask multiply per diagonal block
  normalize: fast reciprocal + gpsimd partition_broadcast + DVE mul, off
    the critical path via an eager PSUM->SBUF copy
  projection: wproj rows for this core's heads (bf16), interleaved into
    pass 1; partial f32 out to HBM
"""

import sys

sys.path.insert(0, "/opt/trn_rl_repo")

import numpy as np
import ml_dtypes

import concourse.bacc as bacc
import concourse.tile as tile
from concourse import mybir
from concourse.bass_utils import run_bass_kernel_spmd

F32 = mybir.dt.float32
BF16 = mybir.dt.bfloat16
AF = mybir.ActivationFunctionType

T, C, D, H, HKV = 2048, 1024, 64, 16, 4
G = H // HKV  # q heads per kv head
NCC = C // 128  # 8 contraction chunks
NJQ = 4  # tq chunks of 512
TQC = 512
NTK = T // 128  # 16 tk tiles
SCALE = 1.0 / 8.0  # 1/sqrt(D)

_PROG = {}


def _build_program():
    nc = bacc.Bacc()
    xT_d = nc.dram_tensor("xT", [C, T], BF16, kind="ExternalInput")
    w_d = nc.dram_tensor("w_all", [C, 384], BF16, kind="ExternalInput")
    wp_d = nc.dram_tensor("wp", [256, C], BF16, kind="ExternalInput")
    cq_d = nc.dram_tensor("cos_q", [128, T], F32, kind="ExternalInput")
    sq_d = nc.dram_tensor("sin_q", [128, T], F32, kind="ExternalInput")
    mk_d = nc.dram_tensor("masks", [128, 128], BF16, kind="ExternalInput")
    id_d = nc.dram_tensor("identb", [128, 128], F32, kind="ExternalInput")
    out_d = nc.dram_tensor("out_p", [T, C], F32, kind="ExternalOutput")

    swap_mask = [i ^ 1 for i in range(32)]

    with tile.TileContext(nc) as tc:
        with (
            tc.tile_pool(name="const", bufs=1) as const,
            tc.tile_pool(name="big", bufs=1) as big,
            tc.tile_pool(name="xp", bufs=1) as xp,
            tc.tile_pool(name="rope", bufs=1) as ropep,
            tc.tile_pool(name="ptiles", bufs=8) as ppool,
            tc.tile_pool(name="small", bufs=3) as small,
            tc.tile_pool(name="outp", bufs=3) as outp,
        ):
            W_sb = const.tile([128, NCC, 384], BF16, tag="W", name="W_sb")
            cq_sb = const.tile([128, T], F32, tag="cq", name="cq_sb")
            sq_sb = const.tile([128, T], F32, tag="sq", name="sq_sb")
            mk_sb = const.tile([128, 128], BF16, tag="mk", name="mk_sb")
            id_sb = const.tile([128, 128], F32, tag="idb", name="id_sb")
            wp_sb = const.tile([128, 2, C], BF16, tag="wp", name="wp_sb")

            qkv_sb = [big.tile([128, T], F32, tag=f"qkv{m}", name=f"qkv{m}") for m in range(3)]
            qrope = [big.tile([128, T], BF16, tag=f"qr{m}", name=f"qr{m}") for m in range(2)]
            k2 = big.tile([128, T], BF16, tag="k2", name="k2")
            vhat = big.tile([128, NTK, 65], BF16, tag="vhat", name="vhat")
            yn = [big.tile([128, T], BF16, tag=f"yn{m}", name=f"yn{m}") for m in range(2)]

            xts = []
            for cc in range(NCC):
                xt = xp.tile([128, T], BF16, tag=f"x{cc}", name=f"x{cc}")
                nc.sync.dma_start(out=xt[:], in_=xT_d[cc * 128 : (cc + 1) * 128, :])
                nc.sync.dma_start(out=W_sb[:, cc, :], in_=w_d[cc * 128 : (cc + 1) * 128, :])
                xts.append(xt)
            nc.sync.dma_start(out=cq_sb[:], in_=cq_d[:])
            nc.sync.dma_start(out=sq_sb[:], in_=sq_d[:])
            nc.sync.dma_start(out=mk_sb[:], in_=mk_d[:])
            nc.sync.dma_start(out=id_sb[:], in_=id_d[:])
            nc.sync.dma_start(out=wp_sb[:], in_=wp_d[:].rearrange("(n p) m -> p n m", p=128))

            def emit_rope(pt, jqs=None):
                rows = 128 if pt < 2 else 64
                dst = qrope[pt] if pt < 2 else k2
                src = qkv_sb[pt]
                for j in jqs if jqs is not None else range(NJQ):
                    cs = slice(j * TQC, (j + 1) * TQC)
                    shuf = ropep.tile([128, TQC], F32, tag="shuf", name="shuf")
                    prod = ropep.tile([128, TQC], F32, tag="prod", name="prod")
                    nc.vector.stream_shuffle(shuf[:rows, :], src[:rows, cs], mask=swap_mask)
                    nc.vector.tensor_mul(out=shuf[:rows, :], in0=shuf[:rows, :], in1=sq_sb[:rows, cs])
                    nc.vector.tensor_mul(out=prod[:rows, :], in0=src[:rows, cs], in1=cq_sb[:rows, cs])
                    nc.vector.tensor_add(out=dst[:rows, cs], in0=prod[:rows, :], in1=shuf[:rows, :])

            # ---- stage A part 1: KV + Q0 projections, rope, Vhat ----
            with (
                tc.tile_pool(name="psA", bufs=1, space="PSUM") as psA,
                tc.tile_pool(name="psT", bufs=2, space="PSUM") as psT,
            ):
                # warmup: garbage matmuls with no input deps keep the PE busy
                # through the initial DMA window so HAM reaches 2.4GHz before
                # stage A starts (values never read; psum overwritten later)
                wu = ropep.tile([128, 512], BF16, tag="wu", name="wu")
                nc.vector.memset(wu[:], 0.5)
                puw = psT.tile([128, 512], F32, tag="ptr", name="puw")
                for _ in range(24):
                    nc.tensor.matmul(puw[:], lhsT=wu[:, 0:128], rhs=wu[:], start=True, stop=True)
                for mt in (2, 0):
                    pas = [psA.tile([128, TQC], F32, tag=f"pa{j}", name=f"pa{j}") for j in range(NJQ)]
                    for cc in range(NCC):
                        for jq in range(NJQ):
                            nc.tensor.matmul(
                                pas[jq][:],
                                lhsT=W_sb[:, cc, mt * 128 : (mt + 1) * 128],
                                rhs=xts[cc][:, jq * TQC : (jq + 1) * TQC],
                                start=(cc == 0),
                                stop=(cc == NCC - 1),
                            )
                    for jq in range(NJQ):
                        nc.scalar.copy(
                            out=qkv_sb[mt][:, jq * TQC : (jq + 1) * TQC], in_=pas[jq][:]
                        )

                emit_rope(2)  # K first: attention depends on it
                # duplicate K^T into partitions 64:128 (head-pair row groups)
                nc.sync.dma_start(out=k2[64:128, :], in_=k2[0:64, :])
                emit_rope(0, jqs=(3, 2, 1, 0))

                # Vhat: V (t-major) + ones column for softmax sums
                nc.vector.memset(vhat[:, :, 64:65], 1.0)
                for tt in range(NTK):
                    pt_ = psT.tile([128, 64], F32, tag="ptr", name="ptr")
                    nc.tensor.transpose(
                        pt_[:],
                        qkv_sb[2][64:128, tt * 128 : (tt + 1) * 128],
                        id_sb[64:128, 0:64],
                    )
                    nc.vector.tensor_copy(out=vhat[:, tt, 0:64], in_=pt_[:])

            def attention_pass(hp, psS, psY, filler, boundary, jq_order=tuple(range(NJQ))):
                """One head-pair pass. filler() emits one unit of extra PE work
                (stage-A Q1 / projection) per group to keep the PE dense;
                boundary(jq) runs after each chunk before the normalize."""
                for jq in jq_order:
                    nik = 4 * jq + 4
                    pys = [psY.tile([65, TQC], F32, tag="py", name="py") for _ in range(2)]
                    for ika in range(0, nik, 2):
                        iks = (ika, ika + 1)
                        filler()
                        qt = qrope[hp]
                        los = []
                        for gi, ik in enumerate(iks):
                            s = ik - 4 * jq
                            los.append(max(s, 0) * 128)
                        ps_gs = [
                            psS.tile([128, 2, TQC], F32, tag=f"ps_g{hh}", name=f"ps_g{hh}")
                            for hh in range(2)
                        ]
                        # alternate row groups (hh base 0 / 64) so consecutive
                        # half-array S matmuls overlap in the PE array
                        for gi, ik in enumerate(iks):
                            lo = los[gi]
                            for hh in range(2):
                                base = hh * 64
                                nc.tensor.matmul(
                                    ps_gs[hh][:, gi, lo:TQC],
                                    lhsT=k2[base : base + 64, ik * 128 : (ik + 1) * 128],
                                    rhs=qt[base : base + 64, jq * TQC + lo : (jq + 1) * TQC],
                                    start=True,
                                    stop=True,
                                )
                        mlo = min(los)
                        ptiles = []
                        for hh in range(2):
                            ptile = ppool.tile([128, 2, TQC], BF16, tag="pt", name="ptile")
                            nc.scalar.activation(
                                out=ptile[:, :, mlo:TQC],
                                in_=ps_gs[hh][:, :, mlo:TQC],
                                func=AF.Exp,
                                scale=SCALE,
                            )
                            ptiles.append(ptile)
                        for hh in range(2):
                            for gi, ik in enumerate(iks):
                                if ik - 4 * jq >= 0:
                                    lo = los[gi]
                                    nc.vector.tensor_mul(
                                        out=ptiles[hh][:, gi, lo : lo + 128],
                                        in0=ptiles[hh][:, gi, lo : lo + 128],
                                        in1=mk_sb[:, 0:128],
                                    )
                        for hh in range(2):
                            for gi, ik in enumerate(iks):
                                lo = los[gi]
                                nc.tensor.matmul(
                                    pys[hh][:, lo:TQC],
                                    lhsT=vhat[:, ik, :],
                                    rhs=ptiles[hh][:, gi, lo:TQC],
                                    start=(ik == 0),
                                    stop=(ik == nik - 1),
                                )
                    boundary(jq)
                    for hh in range(2):
                        # eager copy frees the PSUM accumulator; normalize runs
                        # off the critical path
                        ybuf = small.tile([65, TQC], F32, tag="ybuf", name="ybuf")
                        nc.vector.tensor_copy(out=ybuf[:], in_=pys[hh][:])
                        srow = small.tile([1, TQC], F32, tag="srow", name="srow")
                        nc.vector.tensor_copy(out=srow[:], in_=pys[hh][64:65, :])
                        rinv = small.tile([1, TQC], F32, tag="rinv", name="rinv")
                        nc.vector.reciprocal_approx_fast(out=rinv[:], in_=srow[:])
                        rb = small.tile([64, TQC], F32, tag="rb", name="rb")
                        nc.gpsimd.partition_broadcast(rb[:], rinv[:])
                        nc.vector.tensor_mul(
                            out=yn[hp][hh * 64 : hh * 64 + 64, jq * TQC : (jq + 1) * TQC],
                            in0=ybuf[0:64, :],
                            in1=rb[:],
                        )

            # ---- pass 0 (heads 0,1) with stage-A Q1 interleaved ----
            with (
                tc.tile_pool(name="psS0", bufs=1, space="PSUM") as psS0,
                tc.tile_pool(name="psY0", bufs=2, space="PSUM") as psY0,
                tc.tile_pool(name="psA2", bufs=2, space="PSUM") as psA2,
            ):

                def a2_gen():
                    for jq2 in range(NJQ):
                        pa = psA2.tile([128, TQC], F32, tag="pa2", name="pa2")
                        for cc in range(NCC):
                            nc.tensor.matmul(
                                pa[:],
                                lhsT=W_sb[:, cc, 128:256],
                                rhs=xts[cc][:, jq2 * TQC : (jq2 + 1) * TQC],
                                start=(cc == 0),
                                stop=(cc == NCC - 1),
                            )
                            yield
                        nc.vector.tensor_copy(
                            out=qkv_sb[1][:, jq2 * TQC : (jq2 + 1) * TQC], in_=pa[:]
                        )
                    emit_rope(1)
                    yield

                gen = a2_gen()

                def filler0():
                    next(gen, None)

                def filler0x2():
                    filler0()
                    filler0()

                # head start for the PE while rope-q0 finishes on DVE
                for _ in range(4):
                    filler0()
                attention_pass(
                    0, psS0, psY0, filler0x2, lambda jq: None, jq_order=(3, 2, 1, 0)
                )
                for _ in gen:
                    pass

            # ---- pass 1 (heads 2,3) with projection interleaved ----
            with (
                tc.tile_pool(name="psS1", bufs=1, space="PSUM") as psS1,
                tc.tile_pool(name="psY1", bufs=2, space="PSUM") as psY1,
                tc.tile_pool(name="psP", bufs=1, space="PSUM") as psP,
            ):

                def emit_proj(pjq):
                    for tt in range(4 * pjq, 4 * pjq + 4):
                        outsb = outp.tile([128, C], F32, tag="osb", name="osb")
                        pps = [psP.tile([128, 512], F32, tag=f"pp{n}", name=f"pp{n}") for n in range(2)]
                        for kk in range(2):
                            for ncol in range(2):
                                nc.tensor.matmul(
                                    pps[ncol][:],
                                    lhsT=yn[kk][:, tt * 128 : (tt + 1) * 128],
                                    rhs=wp_sb[:, kk, ncol * 512 : (ncol + 1) * 512],
                                    start=(kk == 0),
                                    stop=(kk == 1),
                                )
                        for ncol in range(2):
                            nc.vector.tensor_copy(
                                out=outsb[:, ncol * 512 : (ncol + 1) * 512], in_=pps[ncol][:]
                            )
                        nc.sync.dma_start(
                            out=out_d[tt * 128 : (tt + 1) * 128, :], in_=outsb[:]
                        )

                def boundary1(jq):
                    if jq > 0:
                        emit_proj(jq - 1)

                attention_pass(1, psS1, psY1, lambda: None, boundary1)
                emit_proj(NJQ - 1)

    nc.compile()
    return nc


def _host_tables():
    # RoPE tables in interleaved-pair device layout (row j'=2i <-> orig j=i,
    # j'=2i+1 <-> orig j=i+32); sign of the shuffled sin term folded in.
    inv = 1.0 / (10000.0 ** (np.arange(0, D, 2, dtype=np.float64) / D))  # (32,)
    t = np.arange(T, dtype=np.float64)
    fr = np.outer(t, inv)  # (T, 32)
    cos_h = np.cos(fr).T.astype(np.float32)  # (32, T)
    sin_h = np.sin(fr).T.astype(np.float32)
    cosI = np.empty((D, T), np.float32)
    sinI = np.empty((D, T), np.float32)
    cosI[0::2] = cos_h
    cosI[1::2] = cos_h
    sinI[0::2] = -sin_h
    sinI[1::2] = sin_h
    cos_q = np.tile(cosI, (2, 1))
    sin_q = np.tile(sinI, (2, 1))
    # tri mask for the diagonal 128-block: allowed iff tkl <= tql
    tkl = np.arange(128)[:, None]
    tql = np.arange(128)[None, :]
    mask = (tkl <= tql).astype(np.float32).astype(ml_dtypes.bfloat16)
    identb = np.tile(np.eye(64, dtype=np.float32), (2, 2))
    return cos_q, sin_q, mask, identb


def make_in_maps(x, wq, wk, wv, wproj):
    cos_q, sin_q, mask, identb = _host_tables()
    # interleave permutation within each head's 64 cols: perm[2i]=i, perm[2i+1]=i+32
    perm = np.empty(D, np.int64)
    perm[0::2] = np.arange(32)
    perm[1::2] = np.arange(32) + 32
    in_maps = []
    for c in range(8):
        b, h = c // 4, c % 4
        xT = np.ascontiguousarray(x[b].T).astype(ml_dtypes.bfloat16)  # (C, T)
        wq_h = wq[:, h * 256 : (h + 1) * 256].reshape(C, G, D)[:, :, perm].reshape(C, 256)
        wk_h = wk[:, h * 64 : (h + 1) * 64][:, perm]
        wv_h = wv[:, h * 64 : (h + 1) * 64]
        w_all = np.concatenate([wq_h, wk_h, wv_h], axis=1).astype(ml_dtypes.bfloat16)
        wp_h = wproj[h * 256 : (h + 1) * 256, :].astype(ml_dtypes.bfloat16)
        in_maps.append(
            {
                "xT": xT,
                "w_all": w_all,
                "wp": wp_h,
                "cos_q": cos_q,
                "sin_q": sin_q,
                "masks": mask,
                "identb": identb,
            }
        )
    return in_maps


def kernel(x, wq, wk, wv, wproj):
    x = np.asarray(x, dtype=np.float32)
    wq = np.asarray(wq, dtype=np.float32)
    wk = np.asarray(wk, dtype=np.float32)
    wv = np.asarray(wv, dtype=np.float32)
    wproj = np.asarray(wproj, dtype=np.float32)
    B = x.shape[0]

    if "nc" not in _PROG:
        _PROG["nc"] = _build_program()
    nc = _PROG["nc"]

    in_maps = make_in_maps(x, wq, wk, wv, wproj)

    res = run_bass_kernel_spmd(nc, in_maps, list(range(8)))
    out = np.zeros((B, T, C), np.float32)
    for c in range(8):
        out[c // 4] += res.results[c]["out_p"]
    return out

